# revision 22
# baseline (speedup 1.0000x reference)
"""VQ codebook (EuclideanCodebook) Trainium2 kernel.

Computes, for x [B=8, N=2048, D=256] and codebook embed [1, C=8192, D=256]:
    dist[m, c] = -sqrt(max(|x_m|^2 - 2 x_m.e_c + |e_c|^2, 0))
    ind[m]     = argmax_c dist[m, c]          (== argmin squared distance,
                                               first index wins ties)
    quantize[m] = embed[ind[m]]

Sharding: data-parallel over tokens. 8 NeuronCores, 2048 tokens each,
codebook replicated.

Per-core algorithm:
  1. Transpose codebook on the PE (identity matmul) into eT [D, C] in SBUF;
     in the same loop square the transposed PSUM blocks and reduce them with
     a ones-column matmul into a -|e_c|^2 row (so nothing serializes on the
     full eT tensor).
  2. Scores s[m, c] = 2 x.e - |e|^2 with float32r (TF32-rate) matmuls into
     PSUM, drained to SBUF by the scalar engine.
  3. argmax: vector-engine InstMax over each half (overlaps the matmul
     stream), small merge, then one full-width InstMaxIndex for global
     top-8 indices.
  4. float32r rounding differs from the reference fp32 math by ~1e-1 in
     score units, so the top-3 candidates are re-scored exactly: gather
     their code vectors by indirect DMA, fp32 sum((x-e)^2) via GPSIMD
     subtract + scalar-engine Square-with-accumulate, lexicographic
     (distance, index) fold, final gather of the winning vector.
     Top-8/index tiles are buffered per m-tile so the rescore pipeline
     trails the scan pipeline without back-pressure.
"""

import numpy as np

B, N, D, C = 8, 2048, 256, 8192
NCORES = 8
P = 128
CH = 512            # codes per score chunk (one PSUM bank)
NCAND = 4           # exact-rescore candidates


def build_module(m_tokens: int, debug: bool = False):
    """Build the per-core Bass module. m_tokens = tokens handled by the core."""
    import concourse.bass as bass
    import concourse.mybir as mybir
    import concourse.tile as tile
    from concourse import bacc

    f32 = mybir.dt.float32
    f32r = mybir.dt.float32r
    bf16 = mybir.dt.bfloat16
    u32 = mybir.dt.uint32
    i32 = mybir.dt.int32
    AF = mybir.ActivationFunctionType
    OP = mybir.AluOpType

    MT = m_tokens // P          # m-tiles
    CCH = C // CH               # score chunks
    CB = C // P                 # codebook row-blocks for transpose

    nc = bacc.Bacc("TRN2", target_bir_lowering=False, debug=debug)

    x_d = nc.dram_tensor("x", [m_tokens, D], f32, kind="ExternalInput")
    e_d = nc.dram_tensor("embed", [C, D], f32, kind="ExternalInput")
    q_d = nc.dram_tensor("q", [m_tokens, D], f32, kind="ExternalOutput")
    i_d = nc.dram_tensor("ind", [m_tokens, 1], i32, kind="ExternalOutput")

    with tile.TileContext(nc) as tc:
        with (
            tc.tile_pool(name="const", bufs=1) as const_pool,
            tc.tile_pool(name="setup", bufs=3) as setup_pool,
            tc.tile_pool(name="natp", bufs=6) as nat_pool,
            tc.tile_pool(name="scores", bufs=2) as score_pool,
            tc.tile_pool(name="xtiles", bufs=3) as x_pool,
            tc.tile_pool(name="xnat", bufs=7) as xn_pool,
            tc.tile_pool(name="cand", bufs=14) as cand_pool,
            tc.tile_pool(name="dsq", bufs=6) as dsq_pool,
            tc.tile_pool(name="top", bufs=8) as top_pool,
            tc.tile_pool(name="small", bufs=16) as small_pool,
            tc.tile_pool(name="psum_mm", bufs=5, space="PSUM") as psum_mm,
            tc.tile_pool(name="psum_tr", bufs=3, space="PSUM") as psum_tr,
        ):
            identity = const_pool.tile([P, P], f32)
            nc.gpsimd.memset(identity, 0.0)
            nc.gpsimd.affine_select(
                out=identity, in_=identity,
                compare_op=OP.not_equal, fill=1.0, base=0,
                pattern=[[-1, P]], channel_multiplier=1,
            )
            ones_col = const_pool.tile([P, 1], f32r)   # lhsT for e^2 reduction
            ones_row = const_pool.tile([1, P], f32r)   # lhsT for -|e|^2 bias mm
            ones_f32 = const_pool.tile([P, P], f32)
            nc.vector.memset(ones_f32, 1.0)
            nc.scalar.copy(ones_col, ones_f32[:, 0:1])
            nc.scalar.copy(ones_row, ones_f32[0:1, :])

            eT0 = const_pool.tile([P, C], bf16)        # e[c, d].T, d in [0,128)
            eT1 = const_pool.tile([P, C], bf16)        # d in [128,256)
            esq_rep = const_pool.tile([P, C], f32r)    # -|e_c|^2 bcast rows
            identity_r = const_pool.tile([P, P], f32r)
            nc.scalar.copy(identity_r, identity)

            # --- codebook transpose + fused |e|^2 ---
            # Per 128-code block: two PE transposes into one PSUM tile; the
            # drains to eT0/eT1 go to scalar/vector; squares read the PSUM
            # blocks (not eT, so nothing serializes on the big tensor) into
            # per-chunk sq tiles; a ones-column matmul then reduces each
            # 512-chunk of squares into the -|e|^2 row.
            for ch in range(CCH):
                sq0 = setup_pool.tile([P, CH], f32r, tag="sq0")
                sq1 = setup_pool.tile([P, CH], f32r, tag="sq1")
                for i in range(4):
                    cb = ch * 4 + i
                    nat = nat_pool.tile([P, D], f32, tag="nat")
                    nc.sync.dma_start(nat, e_d[cb * P:(cb + 1) * P, :])
                    pst = psum_tr.tile([P, D], f32)
                    nc.tensor.transpose(pst[:, 0:P], nat[:, 0:P], identity)
                    nc.tensor.transpose(pst[:, P:D], nat[:, P:D], identity)
                    bsl = slice(cb * P, (cb + 1) * P)
                    csl = slice(i * P, (i + 1) * P)
                    nc.vector.tensor_copy(eT0[:, bsl], pst[:, 0:P])
                    nc.vector.tensor_copy(eT1[:, bsl], pst[:, P:D])
                    nc.scalar.square(sq0[:, csl], pst[:, 0:P])
                    nc.scalar.square(sq1[:, csl], pst[:, P:D])
                sl = slice(ch * CH, (ch + 1) * CH)
                pse = psum_mm.tile([1, CH], f32, tag="ps")
                nc.tensor.matmul(pse, lhsT=ones_col[:], rhs=sq0[:],
                                 start=True, stop=False)
                nc.tensor.matmul(pse, lhsT=ones_col[:], rhs=sq1[:],
                                 start=False, stop=True)
                erow = setup_pool.tile([1, CH], f32r, tag="erow")
                nc.scalar.mul(erow, pse, -1.0)
                pbc = psum_mm.tile([P, CH], f32, tag="ps")
                nc.tensor.matmul(pbc, lhsT=ones_row[:], rhs=erow[:],
                                 start=True, stop=True)
                nc.vector.tensor_copy(esq_rep[:, sl], pbc)

            # --- main loop over m-tiles, software-pipelined: the exact
            # rescore + fold of m-tile T is emitted after the scans of
            # m-tile T+1 so the small fold ops never block the big DVE
            # scans in the engine's static order ---
            pending = []

            def emit_rescore(st, flush=False):
                idx8_, xn_, rows_ = st
                dvals = []
                for k in range(NCAND):
                    cand = cand_pool.tile([P, D], f32, tag="cand")
                    nc.gpsimd.indirect_dma_start(
                        out=cand, out_offset=None, in_=e_d[:],
                        in_offset=bass.IndirectOffsetOnAxis(
                            ap=idx8_[:, k:k + 1], axis=0),
                    )
                    diff = dsq_pool.tile([P, D], f32, tag="diff")
                    sub_eng = nc.vector if flush else nc.gpsimd
                    sub_eng.tensor_sub(diff, xn_, cand)
                    sqs = dsq_pool.tile([P, D], f32, tag="sqs")
                    dk_ = small_pool.tile([P, 1], f32, tag="dv")
                    nc.scalar.activation(sqs, diff, AF.Square, accum_out=dk_)
                    dvals.append(dk_)

                bestd = dvals[0]
                besti = small_pool.tile([P, 1], u32, tag="bi")
                nc.vector.tensor_copy(besti, idx8_[:, 0:1])
                for k in range(1, NCAND):
                    lt = small_pool.tile([P, 1], u32, tag="lt")
                    nc.vector.tensor_tensor(out=lt, in0=dvals[k], in1=bestd,
                                            op=OP.is_lt)
                    eq = small_pool.tile([P, 1], u32, tag="eq")
                    nc.vector.tensor_tensor(out=eq, in0=dvals[k], in1=bestd,
                                            op=OP.is_equal)
                    il = small_pool.tile([P, 1], u32, tag="il")
                    nc.vector.tensor_tensor(out=il, in0=idx8_[:, k:k + 1],
                                            in1=besti, op=OP.is_lt)
                    nc.vector.tensor_tensor(out=eq, in0=eq, in1=il, op=OP.min)
                    nc.vector.tensor_tensor(out=lt, in0=lt, in1=eq, op=OP.max)
                    nc.vector.copy_predicated(bestd, lt, dvals[k])
                    nc.vector.copy_predicated(besti, lt, idx8_[:, k:k + 1])

                qv = cand_pool.tile([P, D], f32, tag="qv")
                nc.gpsimd.indirect_dma_start(
                    out=qv, out_offset=None, in_=e_d[:],
                    in_offset=bass.IndirectOffsetOnAxis(ap=besti[:], axis=0),
                )
                nc.sync.dma_start(q_d[rows_, :], qv)
                nc.sync.dma_start(i_d[rows_, :], besti[:].bitcast(i32))

            for mt in range(MT):
                rows = slice(mt * P, (mt + 1) * P)
                xn = xn_pool.tile([P, D], f32, tag="xn")
                nc.sync.dma_start(xn, x_d[rows, :])
                xt = x_pool.tile([P, D], bf16, tag="xt")   # 2*x^T, d-major
                pst = psum_tr.tile([P, D], f32)
                nc.tensor.transpose(pst[:, 0:P], xn[:, 0:P], identity)
                nc.tensor.transpose(pst[:, P:D], xn[:, P:D], identity)
                nc.scalar.mul(xt[:, 0:P], pst[:, 0:P], 2.0)
                nc.scalar.mul(xt[:, P:D], pst[:, P:D], 2.0)

                sc = score_pool.tile([P, C], f32)
                for ch in range(CCH):
                    sl = slice(ch * CH, (ch + 1) * CH)
                    ps = psum_mm.tile([P, CH], f32, tag="ps")
                    nc.tensor.matmul(ps, lhsT=identity_r[:],
                                     rhs=esq_rep[:, sl],
                                     start=True, stop=False)
                    nc.tensor.matmul(ps, lhsT=xt[:, 0:P], rhs=eT0[:, sl],
                                     start=False, stop=False)
                    nc.tensor.matmul(ps, lhsT=xt[:, P:D], rhs=eT1[:, sl],
                                     start=False, stop=True)
                    nc.scalar.copy(sc[:, sl], ps)

                # top-8 values: halves overlap the matmul stream
                h8a = small_pool.tile([P, 8], f32, tag="h8a")
                nc.vector.max(out=h8a, in_=sc[:, 0:C // 2])
                h8b = small_pool.tile([P, 8], f32, tag="h8b")
                nc.vector.max(out=h8b, in_=sc[:, C // 2:C])
                v16 = small_pool.tile([P, 16], f32, tag="v16")
                nc.scalar.copy(v16[:, 0:8], h8a)
                nc.scalar.copy(v16[:, 8:16], h8b)
                top8 = top_pool.tile([P, 8], f32, tag="top8")
                nc.vector.max(out=top8, in_=v16)
                idx8 = top_pool.tile([P, 8], u32, tag="idx8")
                nc.vector.max_index(out=idx8, in_max=top8, in_values=sc)

                pending.append((idx8, xn, rows))
                lag = 2 if mt < MT - 1 else 1
                while len(pending) > lag:
                    emit_rescore(pending.pop(0))

            for st in pending:
                emit_rescore(st, flush=True)

    nc.compile()
    return nc


_MODULE_CACHE = {}


def _get_module(m_tokens):
    if m_tokens not in _MODULE_CACHE:
        _MODULE_CACHE[m_tokens] = build_module(m_tokens)
    return _MODULE_CACHE[m_tokens]


def kernel(x: np.ndarray, embed: np.ndarray):
    """Full-input entry: x [B, N, D], embed [1, C, D] ->
    (quantize [B, N, D] f32, ind [B, N] int32)."""
    from concourse.bass_utils import run_bass_kernel_spmd

    x = np.asarray(x, dtype=np.float32)
    e2 = np.asarray(embed, dtype=np.float32).reshape(C, D)

    m_total = B * N
    m_per = m_total // NCORES
    xf = x.reshape(m_total, D)

    nc = _get_module(m_per)
    in_maps = [
        {"x": np.ascontiguousarray(xf[i * m_per:(i + 1) * m_per]), "embed": e2}
        for i in range(NCORES)
    ]
    res = run_bass_kernel_spmd(nc, in_maps, core_ids=list(range(NCORES))).results

    q = np.concatenate([r["q"] for r in res], axis=0).reshape(B, N, D)
    ind = np.concatenate([r["ind"] for r in res], axis=0).reshape(B, N)
    return q.astype(np.float32), ind.astype(np.int32)


# revision 23
# speedup vs baseline: 1.0306x; 1.0306x over previous
"""VQ codebook (EuclideanCodebook) Trainium2 kernel.

Computes, for x [B=8, N=2048, D=256] and codebook embed [1, C=8192, D=256]:
    dist[m, c] = -sqrt(max(|x_m|^2 - 2 x_m.e_c + |e_c|^2, 0))
    ind[m]     = argmax_c dist[m, c]          (== argmin squared distance,
                                               first index wins ties)
    quantize[m] = embed[ind[m]]

Sharding: data-parallel over tokens. 8 NeuronCores, 2048 tokens each,
codebook replicated.

Per-core algorithm:
  1. Transpose codebook on the PE (identity matmul) into eT [D, C] in SBUF;
     in the same loop square the transposed PSUM blocks and reduce them with
     a ones-column matmul into a -|e_c|^2 row (so nothing serializes on the
     full eT tensor).
  2. Scores s[m, c] = 2 x.e - |e|^2 with float32r (TF32-rate) matmuls into
     PSUM, drained to SBUF by the scalar engine.
  3. argmax: vector-engine InstMax over each half (overlaps the matmul
     stream), small merge, then one full-width InstMaxIndex for global
     top-8 indices.
  4. float32r rounding differs from the reference fp32 math by ~1e-1 in
     score units, so the top-3 candidates are re-scored exactly: gather
     their code vectors by indirect DMA, fp32 sum((x-e)^2) via GPSIMD
     subtract + scalar-engine Square-with-accumulate, lexicographic
     (distance, index) fold, final gather of the winning vector.
     Top-8/index tiles are buffered per m-tile so the rescore pipeline
     trails the scan pipeline without back-pressure.
"""

import numpy as np

B, N, D, C = 8, 2048, 256, 8192
NCORES = 8
P = 128
CH = 512            # codes per score chunk (one PSUM bank)
NCAND = 4           # exact-rescore candidates


def build_module(m_tokens: int, debug: bool = False):
    """Build the per-core Bass module. m_tokens = tokens handled by the core."""
    import concourse.bass as bass
    import concourse.mybir as mybir
    import concourse.tile as tile
    from concourse import bacc

    f32 = mybir.dt.float32
    f32r = mybir.dt.float32r
    bf16 = mybir.dt.bfloat16
    u32 = mybir.dt.uint32
    i32 = mybir.dt.int32
    AF = mybir.ActivationFunctionType
    OP = mybir.AluOpType

    MT = m_tokens // P          # m-tiles
    CCH = C // CH               # score chunks
    CB = C // P                 # codebook row-blocks for transpose

    nc = bacc.Bacc("TRN2", target_bir_lowering=False, debug=debug)

    x_d = nc.dram_tensor("x", [m_tokens, D], f32, kind="ExternalInput")
    e_d = nc.dram_tensor("embed", [C, D], f32, kind="ExternalInput")
    q_d = nc.dram_tensor("q", [m_tokens, D], f32, kind="ExternalOutput")
    i_d = nc.dram_tensor("ind", [m_tokens, 1], i32, kind="ExternalOutput")

    with tile.TileContext(nc) as tc:
        with (
            tc.tile_pool(name="const", bufs=1) as const_pool,
            tc.tile_pool(name="setup", bufs=3) as setup_pool,
            tc.tile_pool(name="natp", bufs=6) as nat_pool,
            tc.tile_pool(name="scores", bufs=2) as score_pool,
            tc.tile_pool(name="xtiles", bufs=3) as x_pool,
            tc.tile_pool(name="xnat", bufs=7) as xn_pool,
            tc.tile_pool(name="cand", bufs=14) as cand_pool,
            tc.tile_pool(name="dsq", bufs=6) as dsq_pool,
            tc.tile_pool(name="top", bufs=8) as top_pool,
            tc.tile_pool(name="small", bufs=16) as small_pool,
            tc.tile_pool(name="psum_mm", bufs=4, space="PSUM") as psum_mm,
            tc.tile_pool(name="psum_tr", bufs=4, space="PSUM") as psum_tr,
        ):
            identity = const_pool.tile([P, P], f32)
            nc.gpsimd.memset(identity, 0.0)
            nc.gpsimd.affine_select(
                out=identity, in_=identity,
                compare_op=OP.not_equal, fill=1.0, base=0,
                pattern=[[-1, P]], channel_multiplier=1,
            )
            ones_col = const_pool.tile([P, 1], f32r)   # lhsT for e^2 reduction
            ones_row = const_pool.tile([1, P], f32r)   # lhsT for -|e|^2 bias mm
            ones_f32 = const_pool.tile([P, P], f32)
            nc.vector.memset(ones_f32, 1.0)
            nc.scalar.copy(ones_col, ones_f32[:, 0:1])
            nc.scalar.copy(ones_row, ones_f32[0:1, :])

            eT0 = const_pool.tile([P, C], bf16)        # e[c, d].T, d in [0,128)
            eT1 = const_pool.tile([P, C], bf16)        # d in [128,256)
            esq_rep = const_pool.tile([P, C], f32r)    # -|e_c|^2 bcast rows
            identity_r = const_pool.tile([P, P], f32r)
            nc.scalar.copy(identity_r, identity)

            # --- codebook transpose + fused |e|^2 ---
            # Per 128-code block: two PE transposes into one PSUM tile; the
            # drains to eT0/eT1 go to scalar/vector; squares read the PSUM
            # blocks (not eT, so nothing serializes on the big tensor) into
            # per-chunk sq tiles; a ones-column matmul then reduces each
            # 512-chunk of squares into the -|e|^2 row.
            for ch in range(CCH):
                sq0 = setup_pool.tile([P, CH], f32r, tag="sq0")
                sq1 = setup_pool.tile([P, CH], f32r, tag="sq1")
                for i in range(4):
                    cb = ch * 4 + i
                    nat = nat_pool.tile([P, D], f32, tag="nat")
                    nc.sync.dma_start(nat, e_d[cb * P:(cb + 1) * P, :])
                    pst = psum_tr.tile([P, D], f32)
                    nc.tensor.transpose(pst[:, 0:P], nat[:, 0:P], identity)
                    nc.tensor.transpose(pst[:, P:D], nat[:, P:D], identity)
                    bsl = slice(cb * P, (cb + 1) * P)
                    csl = slice(i * P, (i + 1) * P)
                    nc.vector.tensor_copy(eT0[:, bsl], pst[:, 0:P])
                    nc.vector.tensor_copy(eT1[:, bsl], pst[:, P:D])
                    nc.scalar.square(sq0[:, csl], pst[:, 0:P])
                    nc.scalar.square(sq1[:, csl], pst[:, P:D])
                sl = slice(ch * CH, (ch + 1) * CH)
                pse = psum_mm.tile([1, CH], f32, tag="ps")
                nc.tensor.matmul(pse, lhsT=ones_col[:], rhs=sq0[:],
                                 start=True, stop=False)
                nc.tensor.matmul(pse, lhsT=ones_col[:], rhs=sq1[:],
                                 start=False, stop=True)
                erow = setup_pool.tile([1, CH], f32r, tag="erow")
                nc.scalar.mul(erow, pse, -1.0)
                pbc = psum_mm.tile([P, CH], f32, tag="ps")
                nc.tensor.matmul(pbc, lhsT=ones_row[:], rhs=erow[:],
                                 start=True, stop=True)
                nc.vector.tensor_copy(esq_rep[:, sl], pbc)

            # --- main loop over m-tiles, software-pipelined: the exact
            # rescore + fold of m-tile T is emitted after the scans of
            # m-tile T+1 so the small fold ops never block the big DVE
            # scans in the engine's static order ---
            pending = []

            def emit_rescore(st, flush=False):
                idx8_, xn_, rows_ = st
                dvals = []
                for k in range(NCAND):
                    cand = cand_pool.tile([P, D], f32, tag="cand")
                    nc.gpsimd.indirect_dma_start(
                        out=cand, out_offset=None, in_=e_d[:],
                        in_offset=bass.IndirectOffsetOnAxis(
                            ap=idx8_[:, k:k + 1], axis=0),
                    )
                    diff = dsq_pool.tile([P, D], f32, tag="diff")
                    sub_eng = nc.vector if flush else nc.gpsimd
                    sub_eng.tensor_sub(diff, xn_, cand)
                    sqs = dsq_pool.tile([P, D], f32, tag="sqs")
                    dk_ = small_pool.tile([P, 1], f32, tag="dv")
                    nc.scalar.activation(sqs, diff, AF.Square, accum_out=dk_)
                    dvals.append(dk_)

                bestd = dvals[0]
                besti = small_pool.tile([P, 1], u32, tag="bi")
                nc.vector.tensor_copy(besti, idx8_[:, 0:1])
                for k in range(1, NCAND):
                    lt = small_pool.tile([P, 1], u32, tag="lt")
                    nc.vector.tensor_tensor(out=lt, in0=dvals[k], in1=bestd,
                                            op=OP.is_lt)
                    eq = small_pool.tile([P, 1], u32, tag="eq")
                    nc.vector.tensor_tensor(out=eq, in0=dvals[k], in1=bestd,
                                            op=OP.is_equal)
                    il = small_pool.tile([P, 1], u32, tag="il")
                    nc.vector.tensor_tensor(out=il, in0=idx8_[:, k:k + 1],
                                            in1=besti, op=OP.is_lt)
                    nc.vector.tensor_tensor(out=eq, in0=eq, in1=il, op=OP.min)
                    nc.vector.tensor_tensor(out=lt, in0=lt, in1=eq, op=OP.max)
                    nc.vector.copy_predicated(bestd, lt, dvals[k])
                    nc.vector.copy_predicated(besti, lt, idx8_[:, k:k + 1])

                qv = cand_pool.tile([P, D], f32, tag="qv")
                nc.gpsimd.indirect_dma_start(
                    out=qv, out_offset=None, in_=e_d[:],
                    in_offset=bass.IndirectOffsetOnAxis(ap=besti[:], axis=0),
                )
                nc.sync.dma_start(q_d[rows_, :], qv)
                nc.sync.dma_start(i_d[rows_, :], besti[:].bitcast(i32))

            for mt in range(MT):
                rows = slice(mt * P, (mt + 1) * P)
                xn = xn_pool.tile([P, D], f32, tag="xn")
                nc.sync.dma_start(xn, x_d[rows, :])
                xt = x_pool.tile([P, D], bf16, tag="xt")   # 2*x^T, d-major
                pst = psum_tr.tile([P, D], f32)
                nc.tensor.transpose(pst[:, 0:P], xn[:, 0:P], identity)
                nc.tensor.transpose(pst[:, P:D], xn[:, P:D], identity)
                nc.scalar.mul(xt[:, 0:P], pst[:, 0:P], 2.0)
                nc.scalar.mul(xt[:, P:D], pst[:, P:D], 2.0)

                sc = score_pool.tile([P, C], f32)
                for ch in range(CCH):
                    sl = slice(ch * CH, (ch + 1) * CH)
                    ps = psum_mm.tile([P, CH], f32, tag="ps")
                    nc.tensor.matmul(ps, lhsT=identity_r[:],
                                     rhs=esq_rep[:, sl],
                                     start=True, stop=False)
                    nc.tensor.matmul(ps, lhsT=xt[:, 0:P], rhs=eT0[:, sl],
                                     start=False, stop=False)
                    nc.tensor.matmul(ps, lhsT=xt[:, P:D], rhs=eT1[:, sl],
                                     start=False, stop=True)
                    nc.scalar.copy(sc[:, sl], ps)

                # top-8 values: halves overlap the matmul stream
                h8a = small_pool.tile([P, 8], f32, tag="h8a")
                nc.vector.max(out=h8a, in_=sc[:, 0:C // 2])
                h8b = small_pool.tile([P, 8], f32, tag="h8b")
                nc.vector.max(out=h8b, in_=sc[:, C // 2:C])
                v16 = small_pool.tile([P, 16], f32, tag="v16")
                nc.scalar.copy(v16[:, 0:8], h8a)
                nc.scalar.copy(v16[:, 8:16], h8b)
                top8 = top_pool.tile([P, 8], f32, tag="top8")
                nc.vector.max(out=top8, in_=v16)
                idx8 = top_pool.tile([P, 8], u32, tag="idx8")
                nc.vector.max_index(out=idx8, in_max=top8, in_values=sc)

                pending.append((idx8, xn, rows))
                lag = 2 if mt < MT - 1 else 1
                while len(pending) > lag:
                    emit_rescore(pending.pop(0))

            for st in pending:
                emit_rescore(st, flush=True)

    nc.compile()
    return nc


_MODULE_CACHE = {}


def _get_module(m_tokens):
    if m_tokens not in _MODULE_CACHE:
        _MODULE_CACHE[m_tokens] = build_module(m_tokens)
    return _MODULE_CACHE[m_tokens]


def kernel(x: np.ndarray, embed: np.ndarray):
    """Full-input entry: x [B, N, D], embed [1, C, D] ->
    (quantize [B, N, D] f32, ind [B, N] int32)."""
    from concourse.bass_utils import run_bass_kernel_spmd

    x = np.asarray(x, dtype=np.float32)
    e2 = np.asarray(embed, dtype=np.float32).reshape(C, D)

    m_total = B * N
    m_per = m_total // NCORES
    xf = x.reshape(m_total, D)

    nc = _get_module(m_per)
    in_maps = [
        {"x": np.ascontiguousarray(xf[i * m_per:(i + 1) * m_per]), "embed": e2}
        for i in range(NCORES)
    ]
    res = run_bass_kernel_spmd(nc, in_maps, core_ids=list(range(NCORES))).results

    q = np.concatenate([r["q"] for r in res], axis=0).reshape(B, N, D)
    ind = np.concatenate([r["ind"] for r in res], axis=0).reshape(B, N)
    return q.astype(np.float32), ind.astype(np.int32)


# revision 24
# speedup vs baseline: 1.0441x; 1.0132x over previous
"""VQ codebook (EuclideanCodebook) Trainium2 kernel.

Computes, for x [B=8, N=2048, D=256] and codebook embed [1, C=8192, D=256]:
    dist[m, c] = -sqrt(max(|x_m|^2 - 2 x_m.e_c + |e_c|^2, 0))
    ind[m]     = argmax_c dist[m, c]          (== argmin squared distance,
                                               first index wins ties)
    quantize[m] = embed[ind[m]]

Sharding: data-parallel over tokens. 8 NeuronCores, 2048 tokens each,
codebook replicated.

Per-core algorithm:
  1. Transpose codebook on the PE (identity matmul) into bf16 eT [D, C] in
     SBUF; in the same loop square the transposed PSUM blocks and reduce
     them with a ones-column matmul into a replicated -|e_c|^2 tile.
  2. Scores s[m, c] = 2 x.e - |e|^2: per 512-code PSUM chunk, an
     identity-lhsT fp32r matmul injects the bias, then two bf16 matmuls
     contract over the 256 dims; the scalar engine drains PSUM to SBUF.
  3. argmax: vector-engine InstMax over each half (overlaps the matmul
     stream), small merge, then one full-width InstMaxIndex for global
     top-8 indices (first-index tie-break, matching jnp.argmax).
  4. bf16/fp32r rounding differs from the reference fp32 math by ~0.2 in
     score units, so the top-4 candidates are re-scored exactly: gather
     their code vectors by indirect DMA, fp32 sum((x-e)^2) via GPSIMD
     subtract + scalar-engine Square-with-accumulate, lexicographic
     (distance, index) fold on uint32 indices, final gather of the winner.
     The rescore is emitted two m-tiles behind the scans so its small ops
     never head-of-line-block the big vector-engine scans.
"""

import numpy as np

B, N, D, C = 8, 2048, 256, 8192
NCORES = 8
P = 128
CH = 512            # codes per score chunk (one PSUM bank)
NCAND = 4           # exact-rescore candidates


def build_module(m_tokens: int, debug: bool = False):
    """Build the per-core Bass module. m_tokens = tokens handled by the core."""
    import concourse.bass as bass
    import concourse.mybir as mybir
    import concourse.tile as tile
    from concourse import bacc

    f32 = mybir.dt.float32
    f32r = mybir.dt.float32r
    bf16 = mybir.dt.bfloat16
    u32 = mybir.dt.uint32
    i32 = mybir.dt.int32
    AF = mybir.ActivationFunctionType
    OP = mybir.AluOpType

    MT = m_tokens // P          # m-tiles
    CCH = C // CH               # score chunks
    CB = C // P                 # codebook row-blocks for transpose

    nc = bacc.Bacc("TRN2", target_bir_lowering=False, debug=debug)

    x_d = nc.dram_tensor("x", [m_tokens, D], f32, kind="ExternalInput")
    e_d = nc.dram_tensor("embed", [C, D], f32, kind="ExternalInput")
    q_d = nc.dram_tensor("q", [m_tokens, D], f32, kind="ExternalOutput")
    i_d = nc.dram_tensor("ind", [m_tokens, 1], i32, kind="ExternalOutput")

    with tile.TileContext(nc) as tc:
        with (
            tc.tile_pool(name="const", bufs=1) as const_pool,
            tc.tile_pool(name="setup", bufs=3) as setup_pool,
            tc.tile_pool(name="natp", bufs=6) as nat_pool,
            tc.tile_pool(name="scores", bufs=2) as score_pool,
            tc.tile_pool(name="xtiles", bufs=3) as x_pool,
            tc.tile_pool(name="xnat", bufs=7) as xn_pool,
            tc.tile_pool(name="cand", bufs=14) as cand_pool,
            tc.tile_pool(name="dsq", bufs=6) as dsq_pool,
            tc.tile_pool(name="top", bufs=8) as top_pool,
            tc.tile_pool(name="small", bufs=16) as small_pool,
            tc.tile_pool(name="psum_mm", bufs=4, space="PSUM") as psum_mm,
            tc.tile_pool(name="psum_tr", bufs=4, space="PSUM") as psum_tr,
        ):
            identity = const_pool.tile([P, P], f32)
            nc.gpsimd.memset(identity, 0.0)
            nc.gpsimd.affine_select(
                out=identity, in_=identity,
                compare_op=OP.not_equal, fill=1.0, base=0,
                pattern=[[-1, P]], channel_multiplier=1,
            )
            ones_col = const_pool.tile([P, 1], f32r)   # lhsT for e^2 reduction
            ones_row = const_pool.tile([1, P], f32r)   # lhsT for -|e|^2 bias mm
            ones_f32 = const_pool.tile([P, P], f32)
            nc.vector.memset(ones_f32, 1.0)
            nc.scalar.copy(ones_col, ones_f32[:, 0:1])
            nc.scalar.copy(ones_row, ones_f32[0:1, :])

            eT0 = const_pool.tile([P, C], bf16)        # e[c, d].T, d in [0,128)
            eT1 = const_pool.tile([P, C], bf16)        # d in [128,256)
            esq_rep = const_pool.tile([P, C], f32r)    # -|e_c|^2 bcast rows
            identity_r = const_pool.tile([P, P], f32r)
            nc.scalar.copy(identity_r, identity)

            # --- codebook transpose + fused |e|^2 ---
            # Per 128-code block: two PE transposes into one PSUM tile; the
            # drains to eT0/eT1 go to scalar/vector; squares read the PSUM
            # blocks (not eT, so nothing serializes on the big tensor) into
            # per-chunk sq tiles; a ones-column matmul then reduces each
            # 512-chunk of squares into the -|e|^2 row.
            for ch in range(CCH):
                sq0 = setup_pool.tile([P, CH], f32r, tag="sq0")
                sq1 = setup_pool.tile([P, CH], f32r, tag="sq1")
                for i in range(4):
                    cb = ch * 4 + i
                    nat = nat_pool.tile([P, D], f32, tag="nat")
                    nc.sync.dma_start(nat, e_d[cb * P:(cb + 1) * P, :])
                    pst = psum_tr.tile([P, D], f32)
                    nc.tensor.transpose(pst[:, 0:P], nat[:, 0:P], identity)
                    nc.tensor.transpose(pst[:, P:D], nat[:, P:D], identity)
                    bsl = slice(cb * P, (cb + 1) * P)
                    csl = slice(i * P, (i + 1) * P)
                    nc.vector.tensor_copy(eT0[:, bsl], pst[:, 0:P])
                    nc.vector.tensor_copy(eT1[:, bsl], pst[:, P:D])
                    nc.scalar.square(sq0[:, csl], pst[:, 0:P])
                    nc.scalar.square(sq1[:, csl], pst[:, P:D])
                sl = slice(ch * CH, (ch + 1) * CH)
                pse = psum_mm.tile([1, CH], f32, tag="ps")
                nc.tensor.matmul(pse, lhsT=ones_col[:], rhs=sq0[:],
                                 start=True, stop=False)
                nc.tensor.matmul(pse, lhsT=ones_col[:], rhs=sq1[:],
                                 start=False, stop=True)
                erow = setup_pool.tile([1, CH], f32r, tag="erow")
                nc.scalar.mul(erow, pse, -1.0)
                pbc = psum_mm.tile([P, CH], f32, tag="ps")
                nc.tensor.matmul(pbc, lhsT=ones_row[:], rhs=erow[:],
                                 start=True, stop=True)
                nc.vector.tensor_copy(esq_rep[:, sl], pbc)

            # --- main loop over m-tiles, software-pipelined: the exact
            # rescore + fold of m-tile T is emitted after the scans of
            # m-tile T+1 so the small fold ops never block the big DVE
            # scans in the engine's static order ---
            pending = []

            def emit_rescore(st, flush=False):
                idx8_, xn_, rows_ = st
                dvals = []
                for k in range(NCAND):
                    cand = cand_pool.tile([P, D], f32, tag="cand")
                    nc.gpsimd.indirect_dma_start(
                        out=cand, out_offset=None, in_=e_d[:],
                        in_offset=bass.IndirectOffsetOnAxis(
                            ap=idx8_[:, k:k + 1], axis=0),
                    )
                    diff = dsq_pool.tile([P, D], f32, tag="diff")
                    sub_eng = nc.vector if flush else nc.gpsimd
                    sub_eng.tensor_sub(diff, xn_, cand)
                    sqs = dsq_pool.tile([P, D], f32, tag="sqs")
                    dk_ = small_pool.tile([P, 1], f32, tag="dv")
                    nc.scalar.activation(sqs, diff, AF.Square, accum_out=dk_)
                    dvals.append(dk_)

                bestd = dvals[0]
                besti = small_pool.tile([P, 1], u32, tag="bi")
                nc.vector.tensor_copy(besti, idx8_[:, 0:1])
                for k in range(1, NCAND):
                    lt = small_pool.tile([P, 1], u32, tag="lt")
                    nc.vector.tensor_tensor(out=lt, in0=dvals[k], in1=bestd,
                                            op=OP.is_lt)
                    eq = small_pool.tile([P, 1], u32, tag="eq")
                    nc.vector.tensor_tensor(out=eq, in0=dvals[k], in1=bestd,
                                            op=OP.is_equal)
                    il = small_pool.tile([P, 1], u32, tag="il")
                    nc.vector.tensor_tensor(out=il, in0=idx8_[:, k:k + 1],
                                            in1=besti, op=OP.is_lt)
                    nc.vector.tensor_tensor(out=eq, in0=eq, in1=il, op=OP.min)
                    nc.vector.tensor_tensor(out=lt, in0=lt, in1=eq, op=OP.max)
                    nc.vector.copy_predicated(bestd, lt, dvals[k])
                    nc.vector.copy_predicated(besti, lt, idx8_[:, k:k + 1])

                qv = cand_pool.tile([P, D], f32, tag="qv")
                nc.gpsimd.indirect_dma_start(
                    out=qv, out_offset=None, in_=e_d[:],
                    in_offset=bass.IndirectOffsetOnAxis(ap=besti[:], axis=0),
                )
                nc.sync.dma_start(q_d[rows_, :], qv)
                nc.sync.dma_start(i_d[rows_, :], besti[:].bitcast(i32))

            for mt in range(MT):
                rows = slice(mt * P, (mt + 1) * P)
                xn = xn_pool.tile([P, D], f32, tag="xn")
                nc.sync.dma_start(xn, x_d[rows, :])
                xt = x_pool.tile([P, D], bf16, tag="xt")   # 2*x^T, d-major
                pst = psum_tr.tile([P, D], f32)
                nc.tensor.transpose(pst[:, 0:P], xn[:, 0:P], identity)
                nc.tensor.transpose(pst[:, P:D], xn[:, P:D], identity)
                nc.scalar.mul(xt[:, 0:P], pst[:, 0:P], 2.0)
                nc.scalar.mul(xt[:, P:D], pst[:, P:D], 2.0)

                sc = score_pool.tile([P, C], f32)
                for ch in range(CCH):
                    sl = slice(ch * CH, (ch + 1) * CH)
                    ps = psum_mm.tile([P, CH], f32, tag="ps")
                    nc.tensor.matmul(ps, lhsT=identity_r[:],
                                     rhs=esq_rep[:, sl],
                                     start=True, stop=False)
                    nc.tensor.matmul(ps, lhsT=xt[:, 0:P], rhs=eT0[:, sl],
                                     start=False, stop=False)
                    nc.tensor.matmul(ps, lhsT=xt[:, P:D], rhs=eT1[:, sl],
                                     start=False, stop=True)
                    nc.scalar.copy(sc[:, sl], ps)

                # top-8 values: halves overlap the matmul stream
                h8a = small_pool.tile([P, 8], f32, tag="h8a")
                nc.vector.max(out=h8a, in_=sc[:, 0:C // 2])
                h8b = small_pool.tile([P, 8], f32, tag="h8b")
                nc.vector.max(out=h8b, in_=sc[:, C // 2:C])
                v16 = small_pool.tile([P, 16], f32, tag="v16")
                nc.scalar.copy(v16[:, 0:8], h8a)
                nc.scalar.copy(v16[:, 8:16], h8b)
                top8 = top_pool.tile([P, 8], f32, tag="top8")
                nc.vector.max(out=top8, in_=v16)
                idx8 = top_pool.tile([P, 8], u32, tag="idx8")
                nc.vector.max_index(out=idx8, in_max=top8, in_values=sc)

                pending.append((idx8, xn, rows))
                lag = 2 if mt < MT - 1 else 1
                while len(pending) > lag:
                    emit_rescore(pending.pop(0))

            for st in pending:
                emit_rescore(st, flush=True)

    nc.compile()
    return nc


_MODULE_CACHE = {}


def _get_module(m_tokens):
    if m_tokens not in _MODULE_CACHE:
        _MODULE_CACHE[m_tokens] = build_module(m_tokens)
    return _MODULE_CACHE[m_tokens]


def kernel(x: np.ndarray, embed: np.ndarray):
    """Full-input entry: x [B, N, D], embed [1, C, D] ->
    (quantize [B, N, D] f32, ind [B, N] int32)."""
    from concourse.bass_utils import run_bass_kernel_spmd

    x = np.asarray(x, dtype=np.float32)
    e2 = np.asarray(embed, dtype=np.float32).reshape(C, D)

    m_total = B * N
    m_per = m_total // NCORES
    xf = x.reshape(m_total, D)

    nc = _get_module(m_per)
    in_maps = [
        {"x": np.ascontiguousarray(xf[i * m_per:(i + 1) * m_per]), "embed": e2}
        for i in range(NCORES)
    ]
    res = run_bass_kernel_spmd(nc, in_maps, core_ids=list(range(NCORES))).results

    q = np.concatenate([r["q"] for r in res], axis=0).reshape(B, N, D)
    ind = np.concatenate([r["ind"] for r in res], axis=0).reshape(B, N)
    return q.astype(np.float32), ind.astype(np.int32)


# revision 25
# speedup vs baseline: 1.0581x; 1.0134x over previous
"""VQ codebook (EuclideanCodebook) Trainium2 kernel.

Computes, for x [B=8, N=2048, D=256] and codebook embed [1, C=8192, D=256]:
    dist[m, c] = -sqrt(max(|x_m|^2 - 2 x_m.e_c + |e_c|^2, 0))
    ind[m]     = argmax_c dist[m, c]          (== argmin squared distance,
                                               first index wins ties)
    quantize[m] = embed[ind[m]]

Sharding: data-parallel over tokens. 8 NeuronCores, 2048 tokens each,
codebook replicated.

Per-core algorithm:
  1. Transpose codebook on the PE (identity matmul) into bf16 eT [D, C] in
     SBUF; in the same loop square the transposed PSUM blocks and reduce
     them with a ones-column matmul into a replicated -|e_c|^2 tile.
  2. Scores s[m, c] = 2 x.e - |e|^2: per 512-code PSUM chunk, an
     identity-lhsT fp32r matmul injects the bias, then two bf16 matmuls
     contract over the 256 dims; the scalar engine drains PSUM to SBUF.
  3. argmax: vector-engine InstMax over each half (overlaps the matmul
     stream), small merge, then one full-width InstMaxIndex for global
     top-8 indices (first-index tie-break, matching jnp.argmax).
  4. bf16/fp32r rounding differs from the reference fp32 math by ~0.2 in
     score units, so the top-4 candidates are re-scored exactly: gather
     their code vectors by indirect DMA, fp32 sum((x-e)^2) via GPSIMD
     subtract + scalar-engine Square-with-accumulate, lexicographic
     (distance, index) fold on uint32 indices, final gather of the winner.
     The rescore is emitted two m-tiles behind the scans so its small ops
     never head-of-line-block the big vector-engine scans.
"""

import numpy as np

B, N, D, C = 8, 2048, 256, 8192
NCORES = 8
P = 128
CH = 512            # codes per score chunk (one PSUM bank)
NCAND = 4           # exact-rescore candidates


def build_module(m_tokens: int, debug: bool = False):
    """Build the per-core Bass module. m_tokens = tokens handled by the core."""
    import concourse.bass as bass
    import concourse.mybir as mybir
    import concourse.tile as tile
    from concourse import bacc

    f32 = mybir.dt.float32
    f32r = mybir.dt.float32r
    bf16 = mybir.dt.bfloat16
    u32 = mybir.dt.uint32
    i32 = mybir.dt.int32
    AF = mybir.ActivationFunctionType
    OP = mybir.AluOpType

    MT = m_tokens // P          # m-tiles
    CCH = C // CH               # score chunks
    CB = C // P                 # codebook row-blocks for transpose

    nc = bacc.Bacc("TRN2", target_bir_lowering=False, debug=debug)

    x_d = nc.dram_tensor("x", [m_tokens, D], f32, kind="ExternalInput")
    e_d = nc.dram_tensor("embed", [C, D], f32, kind="ExternalInput")
    q_d = nc.dram_tensor("q", [m_tokens, D], f32, kind="ExternalOutput")
    i_d = nc.dram_tensor("ind", [m_tokens, 1], i32, kind="ExternalOutput")

    with tile.TileContext(nc) as tc:
        with (
            tc.tile_pool(name="const", bufs=1) as const_pool,
            tc.tile_pool(name="setup", bufs=3) as setup_pool,
            tc.tile_pool(name="natp", bufs=6) as nat_pool,
            tc.tile_pool(name="scores", bufs=2) as score_pool,
            tc.tile_pool(name="xtiles", bufs=3) as x_pool,
            tc.tile_pool(name="xnat", bufs=7) as xn_pool,
            tc.tile_pool(name="cand", bufs=14) as cand_pool,
            tc.tile_pool(name="dsq", bufs=6) as dsq_pool,
            tc.tile_pool(name="qvp", bufs=3) as qv_pool,
            tc.tile_pool(name="top", bufs=8) as top_pool,
            tc.tile_pool(name="small", bufs=16) as small_pool,
            tc.tile_pool(name="psum_mm", bufs=4, space="PSUM") as psum_mm,
            tc.tile_pool(name="psum_tr", bufs=4, space="PSUM") as psum_tr,
        ):
            identity = const_pool.tile([P, P], f32)
            nc.gpsimd.memset(identity, 0.0)
            nc.gpsimd.affine_select(
                out=identity, in_=identity,
                compare_op=OP.not_equal, fill=1.0, base=0,
                pattern=[[-1, P]], channel_multiplier=1,
            )
            ones_col = const_pool.tile([P, 1], f32r)   # lhsT for e^2 reduction
            ones_row = const_pool.tile([1, P], f32r)   # lhsT for -|e|^2 bias mm
            ones_f32 = const_pool.tile([P, P], f32)
            nc.vector.memset(ones_f32, 1.0)
            nc.scalar.copy(ones_col, ones_f32[:, 0:1])
            nc.scalar.copy(ones_row, ones_f32[0:1, :])

            eT0 = const_pool.tile([P, C], bf16)        # e[c, d].T, d in [0,128)
            eT1 = const_pool.tile([P, C], bf16)        # d in [128,256)
            esq_rep = const_pool.tile([P, C], f32r)    # -|e_c|^2 bcast rows
            identity_r = const_pool.tile([P, P], f32r)
            nc.scalar.copy(identity_r, identity)

            # --- codebook transpose + fused |e|^2 ---
            # Per 128-code block: two PE transposes into one PSUM tile; the
            # drains to eT0/eT1 go to scalar/vector; squares read the PSUM
            # blocks (not eT, so nothing serializes on the big tensor) into
            # per-chunk sq tiles; a ones-column matmul then reduces each
            # 512-chunk of squares into the -|e|^2 row.
            for ch in range(CCH):
                sq0 = setup_pool.tile([P, CH], f32r, tag="sq0")
                sq1 = setup_pool.tile([P, CH], f32r, tag="sq1")
                for i in range(4):
                    cb = ch * 4 + i
                    nat = nat_pool.tile([P, D], f32, tag="nat")
                    nc.sync.dma_start(nat, e_d[cb * P:(cb + 1) * P, :])
                    pst = psum_tr.tile([P, D], f32)
                    nc.tensor.transpose(pst[:, 0:P], nat[:, 0:P], identity)
                    nc.tensor.transpose(pst[:, P:D], nat[:, P:D], identity)
                    bsl = slice(cb * P, (cb + 1) * P)
                    csl = slice(i * P, (i + 1) * P)
                    nc.vector.tensor_copy(eT0[:, bsl], pst[:, 0:P])
                    nc.vector.tensor_copy(eT1[:, bsl], pst[:, P:D])
                    nc.scalar.square(sq0[:, csl], pst[:, 0:P])
                    nc.scalar.square(sq1[:, csl], pst[:, P:D])
                sl = slice(ch * CH, (ch + 1) * CH)
                pse = psum_mm.tile([1, CH], f32, tag="ps")
                nc.tensor.matmul(pse, lhsT=ones_col[:], rhs=sq0[:],
                                 start=True, stop=False)
                nc.tensor.matmul(pse, lhsT=ones_col[:], rhs=sq1[:],
                                 start=False, stop=True)
                erow = setup_pool.tile([1, CH], f32r, tag="erow")
                nc.scalar.mul(erow, pse, -1.0)
                pbc = psum_mm.tile([P, CH], f32, tag="ps")
                nc.tensor.matmul(pbc, lhsT=ones_row[:], rhs=erow[:],
                                 start=True, stop=True)
                nc.vector.tensor_copy(esq_rep[:, sl], pbc)

            # --- main loop over m-tiles, software-pipelined: the exact
            # rescore + fold of m-tile T is emitted after the scans of
            # m-tile T+1 so the small fold ops never block the big DVE
            # scans in the engine's static order ---
            pending = []

            def emit_rescore(st, flush=False):
                idx8_, xn_, rows_ = st
                dvals = []
                for k in range(NCAND):
                    cand = cand_pool.tile([P, D], f32, tag="cand")
                    nc.gpsimd.indirect_dma_start(
                        out=cand, out_offset=None, in_=e_d[:],
                        in_offset=bass.IndirectOffsetOnAxis(
                            ap=idx8_[:, k:k + 1], axis=0),
                    )
                    diff = dsq_pool.tile([P, D], f32, tag="diff")
                    sub_eng = nc.vector if flush else nc.gpsimd
                    sub_eng.tensor_sub(diff, xn_, cand)
                    sqs = dsq_pool.tile([P, D], f32, tag="sqs")
                    dk_ = small_pool.tile([P, 1], f32, tag="dv")
                    nc.scalar.activation(sqs, diff, AF.Square, accum_out=dk_)
                    dvals.append(dk_)

                # tournament fold: lexicographic (d, index) min, 2 rounds
                def duel(da, ia, db, ib):
                    # (da, ia) <- winner of (da, ia) vs (db, ib); mutates a
                    lt = small_pool.tile([P, 1], u32, tag="lt")
                    nc.vector.tensor_tensor(out=lt, in0=db, in1=da,
                                            op=OP.is_lt)
                    eq = small_pool.tile([P, 1], u32, tag="eq")
                    nc.vector.tensor_tensor(out=eq, in0=db, in1=da,
                                            op=OP.is_equal)
                    il = small_pool.tile([P, 1], u32, tag="il")
                    nc.vector.tensor_tensor(out=il, in0=ib, in1=ia,
                                            op=OP.is_lt)
                    nc.vector.tensor_tensor(out=eq, in0=eq, in1=il, op=OP.min)
                    nc.vector.tensor_tensor(out=lt, in0=lt, in1=eq, op=OP.max)
                    nc.vector.copy_predicated(da, lt, db)
                    nc.vector.copy_predicated(ia, lt, ib)

                i01 = small_pool.tile([P, 1], u32, tag="i01")
                nc.vector.tensor_copy(i01, idx8_[:, 0:1])
                i23 = small_pool.tile([P, 1], u32, tag="i23")
                nc.vector.tensor_copy(i23, idx8_[:, 2:3])
                duel(dvals[0], i01, dvals[1], idx8_[:, 1:2])
                duel(dvals[2], i23, dvals[3], idx8_[:, 3:4])
                duel(dvals[0], i01, dvals[2], i23)
                besti = i01

                qv = qv_pool.tile([P, D], f32, tag="qv")
                nc.gpsimd.indirect_dma_start(
                    out=qv, out_offset=None, in_=e_d[:],
                    in_offset=bass.IndirectOffsetOnAxis(ap=besti[:], axis=0),
                )
                nc.sync.dma_start(q_d[rows_, :], qv)
                nc.sync.dma_start(i_d[rows_, :], besti[:].bitcast(i32))

            for mt in range(MT):
                rows = slice(mt * P, (mt + 1) * P)
                xn = xn_pool.tile([P, D], f32, tag="xn")
                nc.sync.dma_start(xn, x_d[rows, :])
                xt = x_pool.tile([P, D], bf16, tag="xt")   # 2*x^T, d-major
                pst = psum_tr.tile([P, D], f32)
                nc.tensor.transpose(pst[:, 0:P], xn[:, 0:P], identity)
                nc.tensor.transpose(pst[:, P:D], xn[:, P:D], identity)
                nc.scalar.mul(xt[:, 0:P], pst[:, 0:P], 2.0)
                nc.scalar.mul(xt[:, P:D], pst[:, P:D], 2.0)

                sc = score_pool.tile([P, C], f32)
                for ch in range(CCH):
                    sl = slice(ch * CH, (ch + 1) * CH)
                    ps = psum_mm.tile([P, CH], f32, tag="ps")
                    nc.tensor.matmul(ps, lhsT=identity_r[:],
                                     rhs=esq_rep[:, sl],
                                     start=True, stop=False)
                    nc.tensor.matmul(ps, lhsT=xt[:, 0:P], rhs=eT0[:, sl],
                                     start=False, stop=False)
                    nc.tensor.matmul(ps, lhsT=xt[:, P:D], rhs=eT1[:, sl],
                                     start=False, stop=True)
                    nc.scalar.copy(sc[:, sl], ps)

                # top-8 values: halves overlap the matmul stream
                h8a = small_pool.tile([P, 8], f32, tag="h8a")
                nc.vector.max(out=h8a, in_=sc[:, 0:C // 2])
                h8b = small_pool.tile([P, 8], f32, tag="h8b")
                nc.vector.max(out=h8b, in_=sc[:, C // 2:C])
                v16 = small_pool.tile([P, 16], f32, tag="v16")
                nc.scalar.copy(v16[:, 0:8], h8a)
                nc.scalar.copy(v16[:, 8:16], h8b)
                top8 = top_pool.tile([P, 8], f32, tag="top8")
                nc.vector.max(out=top8, in_=v16)
                idx8 = top_pool.tile([P, 8], u32, tag="idx8")
                nc.vector.max_index(out=idx8, in_max=top8, in_values=sc)

                pending.append((idx8, xn, rows))
                lag = 2 if mt < MT - 1 else 1
                while len(pending) > lag:
                    emit_rescore(pending.pop(0))

            for st in pending:
                emit_rescore(st, flush=True)

    nc.compile()
    return nc


_MODULE_CACHE = {}


def _get_module(m_tokens):
    if m_tokens not in _MODULE_CACHE:
        _MODULE_CACHE[m_tokens] = build_module(m_tokens)
    return _MODULE_CACHE[m_tokens]


def kernel(x: np.ndarray, embed: np.ndarray):
    """Full-input entry: x [B, N, D], embed [1, C, D] ->
    (quantize [B, N, D] f32, ind [B, N] int32)."""
    from concourse.bass_utils import run_bass_kernel_spmd

    x = np.asarray(x, dtype=np.float32)
    e2 = np.asarray(embed, dtype=np.float32).reshape(C, D)

    m_total = B * N
    m_per = m_total // NCORES
    xf = x.reshape(m_total, D)

    nc = _get_module(m_per)
    in_maps = [
        {"x": np.ascontiguousarray(xf[i * m_per:(i + 1) * m_per]), "embed": e2}
        for i in range(NCORES)
    ]
    res = run_bass_kernel_spmd(nc, in_maps, core_ids=list(range(NCORES))).results

    q = np.concatenate([r["q"] for r in res], axis=0).reshape(B, N, D)
    ind = np.concatenate([r["ind"] for r in res], axis=0).reshape(B, N)
    return q.astype(np.float32), ind.astype(np.int32)
